# revision 13
# baseline (speedup 1.0000x reference)
"""Trainium2 Bass kernel for nn_MmbeddingsEncoder (segment_reduce).

Strategy (data-parallel over 8 NeuronCores):
  - rows (N=1e6) sharded 8-way; each core runs the 2-layer MLP on its shard
    (bf16 stationary-weight matmuls on PE),
  - local segment sums+counts via ONE combined GPSIMD scatter_add stream:
    the scatter_add instruction lets each 16-partition group (Q7 core)
    consume its own index stream, so we pack
       {set0,set1} x {row-half A, row-half B}
    into the 128 partitions (32 partitions each, 2 features per channel in
    d-slots, counts in a spare d-slot).  That cuts the serial per-core index
    stream 4x vs. the naive layout.
  - the two half-accumulators are summed exactly with a tiny fp32 matmul,
  - fp32 ReduceScatter over the 8 cores (each core owns 1024 segments), then
    the small dense head (divide, projections, reparameterized sample),
  - host concatenates the 8 output shards.

Host-side work is limited to data-independent layout/dtype transforms
(sharding, padding, transpose, int16 repack).
"""

import numpy as np
import ml_dtypes

from contextlib import ExitStack

from concourse import bass, mybir, tile, bacc
from concourse.bass_utils import run_bass_kernel_spmd
from concourse.masks import make_identity

BF16 = mybir.dt.bfloat16
F32 = mybir.dt.float32
I16 = mybir.dt.int16

# problem constants (hardcoded per contract)
N = 1_000_000
D_IN = 64
H0, H1 = 128, 64
Q = 8192
D = 16
N_CORES = 8

R = N // N_CORES              # rows per core = 125000
RH = R // 2                   # rows per half  = 62500
CHUNK = 4096                  # rows per half per scatter_add call
N_CHUNK = 16
HP = CHUNK * N_CHUNK          # padded rows per half = 65536
RP = 2 * HP                   # padded rows per core = 131072
QS = Q // N_CORES             # q-shard per core = 1024

SUB = 2048                    # xyt DMA subchunk (columns)
MM = 512                      # matmul free-dim slab


def build_program(n_cores=N_CORES, hp=HP, n_chunk=N_CHUNK, q=Q, qs=None):
    """Build the SPMD Bass program."""
    if qs is None:
        qs = q // n_cores
    chunk = hp // n_chunk
    nsub = chunk // SUB
    nmm = SUB // MM

    nc = bacc.Bacc("TRN2", target_bir_lowering=False, debug=False,
                   num_devices=n_cores)

    # ---- I/O ----
    xyt = nc.dram_tensor("xyt", [D_IN + 1, 2 * hp], BF16, kind="ExternalInput")
    # wrapped-int16 index streams: (set, half)
    idsw = {(s, h): nc.dram_tensor(f"idsw{s}{h}", [16, hp // 16], I16,
                                   kind="ExternalInput")
            for s in range(2) for h in range(2)}
    w0 = nc.dram_tensor("w0", [D_IN + 1, H0], BF16, kind="ExternalInput")
    b0 = nc.dram_tensor("b0", [H0, 1], F32, kind="ExternalInput")
    w1e = nc.dram_tensor("w1e", [H0, H1 // 2], BF16, kind="ExternalInput")
    w1o = nc.dram_tensor("w1o", [H0, H1 // 2], BF16, kind="ExternalInput")
    b1e = nc.dram_tensor("b1e", [H1 // 2, 1], F32, kind="ExternalInput")
    b1o = nc.dram_tensor("b1o", [H1 // 2, 1], F32, kind="ExternalInput")
    wm = [nc.dram_tensor(f"wm{s}", [H1, D], F32, kind="ExternalInput") for s in range(2)]
    bm = [nc.dram_tensor(f"bm{s}", [D, 1], F32, kind="ExternalInput") for s in range(2)]
    wv = [nc.dram_tensor(f"wv{s}", [H1, D], F32, kind="ExternalInput") for s in range(2)]
    bv = [nc.dram_tensor(f"bv{s}", [D, 1], F32, kind="ExternalInput") for s in range(2)]
    epst = [nc.dram_tensor(f"epst{s}", [D, qs], F32, kind="ExternalInput")
            for s in range(2)]
    out = nc.dram_tensor("out", [6, qs, D], F32, kind="ExternalOutput")

    AF = mybir.ActivationFunctionType
    OP = mybir.AluOpType

    with tile.TileContext(nc) as tc, ExitStack() as ctx:
        const = ctx.enter_context(tc.tile_pool(name="const", bufs=1))
        acc_pool = ctx.enter_context(tc.tile_pool(name="acc", bufs=1))
        ids_pool = ctx.enter_context(tc.tile_pool(name="ids", bufs=1))
        phase1 = ExitStack()
        xy_pool = phase1.enter_context(tc.tile_pool(name="xy", bufs=3))
        ht_pool = phase1.enter_context(tc.tile_pool(name="ht", bufs=3))
        add_pool = phase1.enter_context(tc.tile_pool(name="addt", bufs=1))
        ps1 = phase1.enter_context(tc.tile_pool(name="ps1", bufs=2, space="PSUM"))
        ps2 = phase1.enter_context(tc.tile_pool(name="ps2", bufs=2, space="PSUM"))

        # ---- constants / weights ----
        w0t = const.tile([D_IN + 1, H0], BF16)
        nc.sync.dma_start(out=w0t[:], in_=w0[:, :])
        b0t = const.tile([H0, 1], F32)
        nc.sync.dma_start(out=b0t[:], in_=b0[:, :])
        w1et = const.tile([H0, H1 // 2], BF16)
        nc.sync.dma_start(out=w1et[:], in_=w1e[:, :])
        w1ot = const.tile([H0, H1 // 2], BF16)
        nc.sync.dma_start(out=w1ot[:], in_=w1o[:, :])
        b1et = const.tile([H1 // 2, 1], F32)
        nc.sync.dma_start(out=b1et[:], in_=b1e[:, :])
        b1ot = const.tile([H1 // 2, 1], F32)
        nc.sync.dma_start(out=b1ot[:], in_=b1o[:, :])
        wmt = [const.tile([H1, D], F32, name=f"wmt{s}") for s in range(2)]
        wvt = [const.tile([H1, D], F32, name=f"wvt{s}") for s in range(2)]
        bmt = [const.tile([D, 1], F32, name=f"bmt{s}") for s in range(2)]
        bvt = [const.tile([D, 1], F32, name=f"bvt{s}") for s in range(2)]
        for s in range(2):
            nc.sync.dma_start(out=wmt[s][:], in_=wm[s][:, :])
            nc.sync.dma_start(out=wvt[s][:], in_=wv[s][:, :])
            nc.sync.dma_start(out=bmt[s][:], in_=bm[s][:, :])
            nc.sync.dma_start(out=bvt[s][:], in_=bv[s][:, :])
        epstt = [const.tile([D, qs], F32, name=f"epstt{s}") for s in range(2)]
        for s in range(2):
            nc.sync.dma_start(out=epstt[s][:], in_=epst[s][:, :])
        ones64 = const.tile([1, H1], F32)
        nc.vector.memset(ones64[:], 1.0)
        ident = const.tile([128, 128], F32)
        make_identity(nc, ident[:])
        # half-sum matrix [64, 32]: vstack(I32, I32), bf16
        sum2 = const.tile([64, 32], BF16)
        nc.vector.tensor_copy(out=sum2[0:32, :], in_=ident[0:32, 0:32])
        nc.vector.tensor_copy(out=sum2[32:64, :], in_=ident[0:32, 0:32])

        # ---- index streams: partition groups (0,1)=s0A (2,3)=s0B (4,5)=s1A
        #      (6,7)=s1B, each 16-partition group a replica of its stream ----
        idst = ids_pool.tile([128, hp // 16], I16)
        for s in range(2):
            for h in range(2):
                for g in range(2):
                    p0 = 16 * (4 * s + 2 * h + g)
                    nc.sync.dma_start(out=idst[p0:p0 + 16, :],
                                      in_=idsw[(s, h)][:, :])

        # ---- accumulator (bf16) [128, q, 4]; partitions 32*(2s+h)+c,
        #      channel c = features {2c, 2c+1} in d-slots {0,1}, counts slot 2 ----
        acc = acc_pool.tile([128, q * 4], BF16)
        nc.vector.memset(acc[:], 0.0)

        # ---- add tiles (manually double buffered; counts preset once) ----
        addts = [add_pool.tile([128, chunk * 4], BF16, name=f"addtile{p}")
                 for p in range(2)]
        for p in range(2):
            nc.vector.memset(addts[p][:], 0.0)
            nc.vector.memset(addts[p][:, 2:chunk * 4:4], 1.0)

        # ---- main loop ----
        for ci in range(n_chunk):
            addt = addts[ci % 2]
            for h in range(2):
                for si in range(nsub):
                    base = h * hp + ci * chunk + si * SUB
                    xt = xy_pool.tile([D_IN + 1, SUB], BF16)
                    nc.sync.dma_start(out=xt[:], in_=xyt[:, base:base + SUB])
                    for mi in range(nmm):
                        t0 = si * SUB + mi * MM  # within chunk
                        o0 = 4 * t0
                        hs = ht_pool.tile([H0, MM], BF16)
                        hp_ = ps1.tile([H0, MM], F32)
                        nc.tensor.matmul(hp_[:], lhsT=w0t[:],
                                         rhs=xt[:, mi * MM:(mi + 1) * MM],
                                         start=True, stop=True)
                        nc.scalar.activation(hs[:], hp_[:], AF.Relu, bias=b0t[:, :])
                        zpe = ps2.tile([H1 // 2, MM], F32, tag="zpe")
                        nc.tensor.matmul(zpe[:], lhsT=w1et[:], rhs=hs[:],
                                         start=True, stop=True)
                        zpo = ps2.tile([H1 // 2, MM], F32, tag="zpo")
                        nc.tensor.matmul(zpo[:], lhsT=w1ot[:], rhs=hs[:],
                                         start=True, stop=True)
                        # write z1 (bias+relu) into both sets' partitions
                        p0 = 32 * h          # set0 partitions
                        p1 = 64 + 32 * h     # set1 partitions
                        nc.vector.tensor_scalar(
                            out=addt[p0:p0 + 32, o0:o0 + 4 * MM:4],
                            in0=zpe[:], scalar1=b1et[:, :], scalar2=0.0,
                            op0=OP.add, op1=OP.max)
                        nc.scalar.activation(
                            addt[p1:p1 + 32, o0:o0 + 4 * MM:4],
                            zpe[:], AF.Relu, bias=b1et[:, :])
                        nc.vector.tensor_scalar(
                            out=addt[p0:p0 + 32, o0 + 1:o0 + 4 * MM:4],
                            in0=zpo[:], scalar1=b1ot[:, :], scalar2=0.0,
                            op0=OP.add, op1=OP.max)
                        nc.scalar.activation(
                            addt[p1:p1 + 32, o0 + 1:o0 + 4 * MM:4],
                            zpo[:], AF.Relu, bias=b1ot[:, :])
            nc.gpsimd.scatter_add(
                in_ap=acc[:, :],
                idxs_ap=idst[:, ci * (chunk // 16):(ci + 1) * (chunk // 16)],
                add_ap=addt[:, :],
                channels=128, num_elems=q, d=4, num_idxs=chunk)

        phase1.close()

        # ---- extraction (sum halves via matmul) + reduce-scatter ----
        head_pool = ctx.enter_context(tc.tile_pool(name="head", bufs=1))
        sx_pool = ctx.enter_context(tc.tile_pool(name="sx", bufs=2))
        psh = ctx.enter_context(tc.tile_pool(name="psh", bufs=2, space="PSUM"))
        rs_in = nc.dram_tensor("rs_in", [n_cores, 2, 65, qs], F32, kind="Internal")
        rs_out = nc.dram_tensor("rs_out", [2, 65, qs], F32, kind="Internal")
        # set1 accumulator lives on partitions 64..127; after set0 is
        # extracted, move it down onto partitions 0..63 (SBUF-SBUF DMA) so
        # the matmul rhs reads from partition base 0.
        for s in range(2):
            if s == 1:
                nc.sync.dma_start(out=acc[0:64, :], in_=acc[64:128, :])
            src = acc[0:64, :]
            for g in range(n_cores):
                ext = sx_pool.tile([32, qs * 4], F32, tag="ext")
                for j in range(qs * 4 // MM):
                    ep = psh.tile([32, MM], F32, tag="ep")
                    nc.tensor.matmul(
                        ep[:], lhsT=sum2[:],
                        rhs=src[:, g * qs * 4 + j * MM:g * qs * 4 + (j + 1) * MM],
                        start=True, stop=True)
                    nc.vector.tensor_copy(out=ext[:, j * MM:(j + 1) * MM], in_=ep[:])
                for j in range(2):
                    nc.sync.dma_start(
                        out=rs_in[g, s, j:64:2, :],
                        in_=ext[:, j:qs * 4:4])
                nc.sync.dma_start(out=rs_in[g, s, 64:65, :],
                                  in_=ext[0:1, 2:qs * 4:4])
        nc.gpsimd.collective_compute(
            "ReduceScatter", OP.add,
            replica_groups=[list(range(n_cores))],
            ins=[rs_in[:, :, :, :]], outs=[rs_out[:, :, :]])

        # ---- head on owned q-shard ----
        stt = head_pool.tile([65, 2 * qs], F32, tag="stt")
        for s in range(2):
            nc.sync.dma_start(out=stt[:, s * qs:(s + 1) * qs], in_=rs_out[s])
        cl = head_pool.tile([1, 2 * qs], F32, tag="cl")
        nc.vector.tensor_scalar_max(cl[:], stt[64:65, :], 1.0)
        rec = head_pool.tile([1, 2 * qs], F32, tag="rec")
        nc.vector.reciprocal(rec[:], cl[:])
        recb = head_pool.tile([H1, 2 * qs], F32, tag="recb")
        for j in range(0, 2 * qs, MM):
            rp_ = psh.tile([H1, MM], F32, tag="recp")
            nc.tensor.matmul(rp_[:], lhsT=ones64[:], rhs=rec[:, j:j + MM],
                             start=True, stop=True)
            nc.vector.tensor_copy(out=recb[:, j:j + MM], in_=rp_[:])
        bt = head_pool.tile([H1, 2 * qs], F32, tag="bt")
        nc.vector.tensor_tensor(out=bt[:], in0=stt[0:64, :], in1=recb[:], op=OP.mult)

        projT = []
        for s in range(2):
            mT = head_pool.tile([D, qs], F32, name=f"mT{s}")
            vT = head_pool.tile([D, qs], F32, name=f"vT{s}")
            for (wt, bt_, dst) in ((wmt[s], bmt[s], mT), (wvt[s], bvt[s], vT)):
                for j in range(0, qs, MM):
                    pp = psh.tile([D, MM], F32, tag="proj")
                    nc.tensor.matmul(pp[:], lhsT=wt[:],
                                     rhs=bt[:, s * qs + j:s * qs + j + MM],
                                     start=True, stop=True)
                    nc.vector.tensor_scalar(out=dst[:, j:j + MM], in0=pp[:],
                                            scalar1=bt_[:, :], scalar2=None,
                                            op0=OP.add)
            projT.append((mT, vT))
        sampT = []
        for s in range(2):
            mT, vT = projT[s]
            e = head_pool.tile([D, qs], F32, name=f"eT{s}")
            nc.scalar.activation(e[:], vT[:], AF.Exp, scale=0.5)
            sm = head_pool.tile([D, qs], F32, name=f"smT{s}")
            nc.vector.tensor_tensor(out=sm[:], in0=e[:], in1=epstt[s][:], op=OP.mult)
            nc.vector.tensor_tensor(out=sm[:], in0=sm[:], in1=mT[:], op=OP.add)
            sampT.append(sm)

        # ---- transpose back to natural layout + output ----
        slabs = [projT[0][0], projT[1][0], projT[0][1], projT[1][1],
                 sampT[0], sampT[1]]
        nt = qs // 128
        ost = head_pool.tile([128, 6 * nt * D], F32, tag="ost")
        for si_, src in enumerate(slabs):
            for t in range(nt):
                tp = psh.tile([128, D], F32, tag="otp")
                nc.tensor.transpose(tp[:], src[:, t * 128:(t + 1) * 128],
                                    ident[0:D, 0:D])
                o = (si_ * nt + t) * D
                nc.vector.tensor_copy(out=ost[:, o:o + D], in_=tp[:])
        for si_ in range(6):
            nc.sync.dma_start(
                out=out[si_].rearrange("(t p) d -> p t d", p=128),
                in_=ost[:, si_ * nt * D:(si_ + 1) * nt * D].rearrange(
                    "p (t d) -> p t d", d=D))

    nc.compile()
    return nc


_CACHE = {}


def _get_program():
    if "nc" not in _CACHE:
        _CACHE["nc"] = build_program()
    return _CACHE["nc"]


def _prep_inputs(X, y, z_ids0, z_ids1, W0, b0, W1, b1,
                 Wm0, bm0, Wv0, bv0, Wm1, bm1, Wv1, bv1, eps0, eps1,
                 n_cores=N_CORES, r=R, hp=HP, qs=QS):
    """Host-side data-independent prep: shard/pad/layout/dtype only."""
    bf16 = ml_dtypes.bfloat16
    rh = r // 2
    xy = np.concatenate([np.asarray(X), np.asarray(y)], axis=1)  # [N, 65]
    xyt_full = np.ascontiguousarray(xy.T.astype(bf16))           # [65, N]

    in_maps = []
    for c in range(n_cores):
        lo = c * r
        m = {}
        xt = np.zeros((D_IN + 1, 2 * hp), dtype=bf16)
        xt[:, :rh] = xyt_full[:, lo:lo + rh]
        xt[:, hp:hp + (r - rh)] = xyt_full[:, lo + rh:lo + r]
        m["xyt"] = xt
        for s, ids in enumerate((z_ids0, z_ids1)):
            idc = np.asarray(ids[lo:lo + r]).astype(np.int16)
            for h in range(2):
                idp = np.full((hp,), -1, dtype=np.int16)
                part = idc[:rh] if h == 0 else idc[rh:]
                idp[:len(part)] = part
                m[f"idsw{s}{h}"] = np.ascontiguousarray(
                    idp.reshape(hp // 16, 16).T)
        m["w0"] = np.asarray(W0).astype(bf16)
        m["b0"] = np.asarray(b0).astype(np.float32).reshape(H0, 1)
        W1np = np.asarray(W1).astype(bf16)
        b1np = np.asarray(b1).astype(np.float32)
        m["w1e"] = np.ascontiguousarray(W1np[:, 0::2])
        m["w1o"] = np.ascontiguousarray(W1np[:, 1::2])
        m["b1e"] = np.ascontiguousarray(b1np[0::2].reshape(H1 // 2, 1))
        m["b1o"] = np.ascontiguousarray(b1np[1::2].reshape(H1 // 2, 1))
        for s, (Wm, bm, Wv, bv, eps) in enumerate(
                ((Wm0, bm0, Wv0, bv0, eps0), (Wm1, bm1, Wv1, bv1, eps1))):
            m[f"wm{s}"] = np.asarray(Wm).astype(np.float32)
            m[f"bm{s}"] = np.asarray(bm).astype(np.float32).reshape(D, 1)
            m[f"wv{s}"] = np.asarray(Wv).astype(np.float32)
            m[f"bv{s}"] = np.asarray(bv).astype(np.float32).reshape(D, 1)
            m[f"epst{s}"] = np.ascontiguousarray(
                np.asarray(eps[c * qs:(c + 1) * qs]).astype(np.float32).T)
        in_maps.append(m)
    return in_maps


def kernel(**inputs):
    nc = _get_program()
    in_maps = _prep_inputs(**inputs)
    res = run_bass_kernel_spmd(nc, in_maps, core_ids=list(range(N_CORES)))
    shards = [res.results[c]["out"] for c in range(N_CORES)]
    return np.concatenate(shards, axis=1).astype(np.float32)


if __name__ == "__main__":
    nc = build_program()
    print("program built OK")
